# revision 23
# baseline (speedup 1.0000x reference)
"""Trainium2 Bass kernel for sparse CausalSelfAttention (8 full heads W=1024,
8 reduced-qk heads W=256), SPMD over 8 NeuronCores.

Sharding: core c -> batch c//4, head-group g=c%4 (full heads 2g,2g+1 and
reduced heads 2g,2g+1). fp16 activations/weights (fp32 PSUM accumulate),
fused per-512-block loop: project block -> attention q-blocks -> c_proj,
c_proj partials DMA'd straight from PSUM; host sums the 4 partials.
"""

import numpy as np

import concourse.bacc as bacc
import concourse.mybir as mybir
from concourse import bass_utils
from concourse.tile import TileContext

# problem constants (hardcoded; kernel.py must be self-contained)
B, T, C = 2, 2048, 1024
HDIM = 64          # full head dim (and v dim of reduced heads)
RDIM = 32          # reduced qk dim
WF, WR = 1024, 256  # windows
QF, QR = 256, 256   # query-block sizes
N_CORES = 8
NK = C // 128       # k-tiles over C contraction
PV_LAG = 2          # software-pipeline depth: PV matmuls lag scores

F32 = mybir.dt.float32
F16 = mybir.dt.float16

# mask offsets d = i0 - kt*128 that need masking: 1.0 where 0 <= d+f-p < W
MASKF_D = [-128, 0, 896, 1024]   # full heads (Q=256, W=1024)
MASKR_D = [256, 128, 0, -128]    # reduced heads (Q=256, W=256)


def _make_mask(nc, dst, d, w):
    """dst[p, f] = 1.0 where 0 <= d + f - p < w else 0.0 (on gpsimd).

    For every offset used here exactly one bound can actually trigger
    (Q=256 tiles), so emit a single affine_select."""
    q = dst.shape[-1]
    nc.gpsimd.memset(dst, 1.0)
    lower_can_fail = d - 127 < 0          # min over tile of d+f-p
    upper_can_fail = d + q - 1 >= w       # max over tile of d+f-p
    assert lower_can_fail != upper_can_fail, (d, w)
    if lower_can_fail:
        nc.gpsimd.affine_select(out=dst, in_=dst,
                                compare_op=mybir.AluOpType.is_ge,
                                fill=0.0, base=d, pattern=[[1, q]],
                                channel_multiplier=-1)
    else:
        nc.gpsimd.affine_select(out=dst, in_=dst,
                                compare_op=mybir.AluOpType.is_ge,
                                fill=0.0, base=w - 1 - d, pattern=[[-1, q]],
                                channel_multiplier=1)


def _emit_body(nc, pools, aps):
    (wpool, xbpool, qkpool, ppool, opool, rpool,
     ps_m, ps_s, ps_y) = pools
    xT, wqkv, wproj, out = aps

    # ---- single merged qkv weight tile (one DMA): cols 0:128 wq | 128:256
    # wk | 256:384 packed wqkr | 384:640 wv ----
    wqkv_sb = wpool.tile([128, NK, 640], F16, tag="wqkv")
    wq_sb = wqkv_sb[:, :, 0:128]
    wk_sb = wqkv_sb[:, :, 128:256]
    wqkr_sb = wqkv_sb[:, :, 256:384]
    wv_sb = wqkv_sb[:, :, 384:640]
    wproj_sb = wpool.tile([128, 2, C], F16, tag="wproj")

    # ---- masks generated on gpsimd (emitted after block-0 projections so
    # they don't block the q/k psum->sbuf copies attention 0 needs) ----
    mf_sb = wpool.tile([128, len(MASKF_D), QF], F16, tag="mf")
    mr_sb = None  # reduced heads are masked in-place via affine_select

    def gen_masks():
        # in first-use order: full d=0,-128 (qb0), then 896/1024
        for d in (0, -128, 896, 1024):
            _make_mask(nc, mf_sb[:, MASKF_D.index(d), :], d, WF)

    # persistent transposed activations [dim-stack, T]
    qTf = qkpool.tile([128, T], F16, tag="qTf")  # rows: hA q (64) | hB q (64)
    kTf = qkpool.tile([128, T], F16, tag="kTf")
    # packed reduced: kTr rows 0:32 krA, 32:64 krB, duplicated at 64:128
    # (matmul lhsT/rhs must share a base partition)
    # qTr rows 0:32 qrA, 32:64 zero | 64:96 zero, 96:128 qrB
    qTr = qkpool.tile([128, T], F16, tag="qTr")
    kTr = qkpool.tile([128, T], F16, tag="kTr")
    nc.gpsimd.memset(qTr[32:64, :], 0.0)
    nc.gpsimd.memset(qTr[64:96, :], 0.0)
    # v values + ones block: [128, T-tile, head, 128] (cols 64:128 = 1.0)
    v_sb = qkpool.tile([128, T // 128, 4, 128], F16, tag="v")
    nc.gpsimd.memset(v_sb[:, :, :, 64:128], 1.0)
    # attention outputs yT (normalized), stacked per pair
    yTf = qkpool.tile([128, T], F16, tag="yTf")
    yTr = qkpool.tile([128, T], F16, tag="yTr")

    xT3 = xT.rearrange("(k p) t -> p k t", p=128)

    def project(tb, xts):
        for w_sb, dsts in (
            (wq_sb, ((slice(0, 128), slice(0, 128), qTf),)),
            (wk_sb, ((slice(0, 128), slice(0, 128), kTf),)),
            (wqkr_sb, ((slice(0, 64), slice(0, 64), kTr),
                       (slice(0, 64), slice(64, 128), kTr),
                       (slice(64, 96), slice(0, 32), qTr),
                       (slice(96, 128), slice(96, 128), qTr))),
        ):
            psum = ps_m.tile([128, 512], F32, tag="m")
            for k in range(NK):
                nc.tensor.matmul(psum[:], w_sb[:, k, :], xts[k],
                                 start=(k == 0), stop=(k == NK - 1))
            sl = slice(tb * 512, (tb + 1) * 512)
            for src_rows, dst_rows, dst in dsts:
                nc.vector.tensor_copy(dst[dst_rows, sl], psum[src_rows, :])
        for tt in range(4):
            gt = tb * 4 + tt  # global T-tile
            psv = ps_m.tile([128, 256], F32, tag="m")
            for k in range(NK):
                nc.tensor.matmul(psv[:], xts[k][:, tt * 128:(tt + 1) * 128],
                                 wv_sb[:, k, :],
                                 start=(k == 0), stop=(k == NK - 1))
            nc.vector.tensor_copy(
                v_sb[:, gt, :, 0:64],
                psv[:].rearrange("p (h d) -> p h d", h=4))

    def attn_block(Q, W, m_sb, mask_d, heads, yT, is_full, qb,
                   mid_cb=None):
        i0 = qb * Q
        kt_lo = max(0, i0 - W + 1) // 128
        kt_hi = (i0 + Q - 1) // 128
        kts = list(range(kt_lo, kt_hi + 1))
        py = ps_y.tile([128, 2, 512], F32, tag="y")
        pend = []  # software pipeline: PV lags scores by PV_LAG k-tiles
        for idx, kt in enumerate(kts):
            d = i0 - kt * 128
            pss = ps_s.tile([128, 2, 512], F32, tag="s")
            ksl = slice(kt * 128, (kt + 1) * 128)
            qsl = slice(i0, i0 + Q)
            if is_full:
                nc.tensor.matmul(pss[:, 0, 0:Q], kTf[0:64, ksl],
                                 qTf[0:64, qsl], start=True, stop=True)
                nc.tensor.matmul(pss[:, 1, 0:Q], kTf[64:128, ksl],
                                 qTf[64:128, qsl], start=True, stop=True)
            else:
                nc.tensor.matmul(pss[:, 0, 0:Q], kTr[0:64, ksl],
                                 qTr[0:64, qsl], start=True, stop=True)
                nc.tensor.matmul(pss[:, 1, 0:Q], kTr[64:128, ksl],
                                 qTr[64:128, qsl], start=True, stop=True)
            p_sb = ppool.tile([128, 2, Q], F16, tag="p")
            nc.scalar.activation(p_sb[:], pss[:, :, 0:Q],
                                 mybir.ActivationFunctionType.Exp)
            if d in mask_d:
                if is_full:
                    mm = m_sb[:, mask_d.index(d), :].rearrange(
                        "p (a q) -> p a q", a=1).broadcast_to([128, 2, Q])
                    nc.vector.tensor_mul(p_sb[:], p_sb[:], mm)
                elif d - 127 < 0:  # lower bound: keep where d + f - p >= 0
                    nc.gpsimd.affine_select(
                        out=p_sb[:], in_=p_sb[:],
                        compare_op=mybir.AluOpType.is_ge, fill=0.0,
                        base=d, pattern=[[0, 2], [1, Q]],
                        channel_multiplier=-1)
                else:  # upper bound: keep where d + f - p < W
                    nc.gpsimd.affine_select(
                        out=p_sb[:], in_=p_sb[:],
                        compare_op=mybir.AluOpType.is_ge, fill=0.0,
                        base=W - 1 - d, pattern=[[0, 2], [-1, Q]],
                        channel_multiplier=1)
            pend.append((p_sb, kt, idx))
            if len(pend) > PV_LAG:
                q0 = pend.pop(0)
                _emit_pv(py, q0[0], q0[1], heads,
                         first=(q0[2] == 0), last=(q0[2] == len(kts) - 1))
        if mid_cb is not None:
            mid_cb()
        for q0 in pend:
            _emit_pv(py, q0[0], q0[1], heads,
                     first=(q0[2] == 0), last=(q0[2] == len(kts) - 1))
        # normalize: yT rows = py[0:64] * reciprocal(denominator rows)
        r_sb = rpool.tile([64, 2, Q], F32, tag="r")
        nc.vector.reciprocal(r_sb[:], py[64:128, :, 0:Q])
        for h, rows in ((0, slice(0, 64)), (1, slice(64, 128))):
            nc.vector.tensor_mul(yT[rows, i0:i0 + Q], py[0:64, h, 0:Q],
                                 r_sb[:, h, :])

    def _emit_pv(py, p_sb, kt, heads, first, last):
        Q = p_sb.shape[-1]
        nc.tensor.matmul(py[:, 0, 0:Q], v_sb[:, kt, heads[0], :],
                         p_sb[:, 0, :], start=first, stop=last)
        nc.tensor.matmul(py[:, 1, 0:Q], v_sb[:, kt, heads[1], :],
                         p_sb[:, 1, :], start=first, stop=last)

    def cproj_pair(qb):
        # c_proj for the two 128-row T-tiles covered by q-block qb
        o_sb = opool.tile([128, 2, C], F16, tag="o")
        for j in range(2):
            tt = 2 * qb + j
            tsl = slice(tt * 128, (tt + 1) * 128)
            for nb in range(2):
                nsl = slice(nb * 512, (nb + 1) * 512)
                pso = ps_m.tile([128, 512], F32, tag="m")
                nc.tensor.matmul(pso[:], yTf[:, tsl], wproj_sb[:, 0, nsl],
                                 start=True, stop=False)
                nc.tensor.matmul(pso[:], yTr[:, tsl], wproj_sb[:, 1, nsl],
                                 start=False, stop=True)
                if nb == 0:
                    nc.scalar.copy(o_sb[:, j, nsl], pso[:])
                else:
                    nc.vector.tensor_copy(o_sb[:, j, nsl], pso[:])
        nc.sync.dma_start(
            out[qb * 256:(qb + 1) * 256, :].rearrange("(j p) m -> p j m",
                                                      p=128), o_sb[:])

    # ---- fused per-512-block loop ----
    xtbs = [None] * 4
    wqkv3 = wqkv.rearrange("(k p) m -> p k m", p=128)
    for tb in range(T // 512):
        sl = slice(tb * 512, (tb + 1) * 512)
        if tb == 0:
            # weights first (one merged DMA), x block 0 in two chunks so
            # the first matmuls can start at the halfway point
            nc.sync.dma_start(wqkv_sb[:, 0:2, :], wqkv3[:, 0:2, :])
            xtb = xbpool.tile([128, NK, 512], F16, tag="xtb")
            nc.scalar.dma_start(xtb[:, 0:2, :], xT3[:, 0:2, sl])
            nc.sync.dma_start(wqkv_sb[:, 2:NK, :], wqkv3[:, 2:NK, :])
            nc.scalar.dma_start(xtb[:, 2:NK, :], xT3[:, 2:NK, sl])
            nc.sync.dma_start(wproj_sb[:],
                              wproj.rearrange("(k p) m -> p k m", p=128))
            xtbs[0] = xtb
        xtb = xtbs[tb]
        xts = [xtb[:, k, :] for k in range(NK)]
        project(tb, xts)
        if tb == 0:
            gen_masks()
        if tb + 1 < 4:  # prefetch next x block behind the projections
            nsl = slice((tb + 1) * 512, (tb + 2) * 512)
            xtb = xbpool.tile([128, NK, 512], F16, tag="xtb")
            nc.scalar.dma_start(xtb[:], xT3[:, :, nsl])
            xtbs[tb + 1] = xtb
        qbs = (2 * tb, 2 * tb + 1)
        attn_block(QF, WF, mf_sb, MASKF_D, (0, 1), yTf, True, qbs[0])
        if tb > 0:
            cproj_pair(qbs[0] - 2)  # needs full+red of 2 q-blocks ago
        attn_block(QF, WF, mf_sb, MASKF_D, (0, 1), yTf, True, qbs[1])
        for sub in range(2):
            qb = qbs[sub]
            if qb >= 1 and sub == 0:
                cproj_pair(qb - 1)
            mid_cb = None
            if qb == 7:
                # last pair: yTf contribution inside red(7)'s score shadow,
                # yTr contribution + store at the very end
                psos = []

                def mid_cb():
                    for j in range(2):
                        tsl = slice((14 + j) * 128, (15 + j) * 128)
                        for nb in range(2):
                            nsl = slice(nb * 512, (nb + 1) * 512)
                            pso = ps_m.tile([128, 512], F32, tag="m")
                            nc.tensor.matmul(pso[:], yTf[:, tsl],
                                             wproj_sb[:, 0, nsl],
                                             start=True, stop=False)
                            psos.append((pso, tsl, nsl, nb))
            attn_block(QR, WR, mr_sb, MASKR_D, (2, 3), yTr, False, qb,
                       mid_cb=mid_cb)
            if qb == 6:
                cproj_pair(6)
    for j in range(2):
        tt = 14 + j
        o_sb = opool.tile([128, C], F16, tag="o2")
        for nb in range(2):
            pso, tsl, nsl, _ = psos[2 * j + nb]
            nc.tensor.matmul(pso[:], yTr[:, tsl], wproj_sb[:, 1, nsl],
                             start=False, stop=True)
            if nb == 0:
                nc.scalar.copy(o_sb[:, nsl], pso[:])
            else:
                nc.vector.tensor_copy(o_sb[:, nsl], pso[:])
            nc.sync.dma_start(out[tt * 128:(tt + 1) * 128, nsl],
                              o_sb[:, nsl])


def _build_nc(reps=1):
    nc = bacc.Bacc(trn_type="TRN2", target_bir_lowering=False, debug=False,
                   num_devices=1)

    xT = nc.dram_tensor("xT", [C, T], F16, kind="ExternalInput").ap()
    wqkv = nc.dram_tensor("wqkv", [C, 640], F16, kind="ExternalInput").ap()
    wproj = nc.dram_tensor("wproj", [256, C], F16, kind="ExternalInput").ap()
    out = nc.dram_tensor("o", [T, C], F16, kind="ExternalOutput").ap()
    aps = (xT, wqkv, wproj, out)

    with TileContext(nc) as tc:
        with (
            tc.tile_pool(name="wpool", bufs=1) as wpool,
            tc.tile_pool(name="xbpool", bufs=2) as xbpool,
            tc.tile_pool(name="qk", bufs=1) as qkpool,
            tc.tile_pool(name="ppool", bufs=PV_LAG + 3) as ppool,
            tc.tile_pool(name="opool", bufs=3) as opool,
            tc.tile_pool(name="rpool", bufs=4) as rpool,
            tc.tile_pool(name="ps_m", bufs=2, space="PSUM") as ps_m,
            tc.tile_pool(name="ps_s", bufs=2, space="PSUM") as ps_s,
            tc.tile_pool(name="ps_y", bufs=1, space="PSUM") as ps_y,
        ):
            pools = (wpool, xbpool, qkpool, ppool, opool, rpool,
                     ps_m, ps_s, ps_y)
            for _ in range(reps):
                _emit_body(nc, pools, aps)

    nc.compile()
    return nc


_NC_CACHE = {}


def _get_nc(reps=1):
    if reps not in _NC_CACHE:
        _NC_CACHE[reps] = _build_nc(reps)
    return _NC_CACHE[reps]


def make_in_maps(x, w_qkv_full, w_qk_red, w_v_red, w_proj):
    x = np.asarray(x, np.float32)
    w_qkv_full = np.asarray(w_qkv_full, np.float32)
    w_qk_red = np.asarray(w_qk_red, np.float32)
    w_v_red = np.asarray(w_v_red, np.float32)
    w_proj = np.asarray(w_proj, np.float32)
    sf = np.float32(1.0 / np.sqrt(HDIM))
    sr = np.float32(1.0 / np.sqrt(RDIM))
    in_maps = []
    for c in range(N_CORES):
        b, g = divmod(c, 4)
        hA, hB = 2 * g, 2 * g + 1
        wq = np.concatenate([w_qkv_full[:, 64 * hA:64 * hA + 64],
                             w_qkv_full[:, 64 * hB:64 * hB + 64]], 1) * sf
        wk = np.concatenate([w_qkv_full[:, 512 + 64 * hA:512 + 64 * hA + 64],
                             w_qkv_full[:, 512 + 64 * hB:512 + 64 * hB + 64]], 1)
        # packed reduced projection: rows 0:32 krA | 32:64 krB | qrA | qrB
        wqkr = np.concatenate(
            [w_qk_red[:, 256 + 32 * hA:256 + 32 * hA + 32],
             w_qk_red[:, 256 + 32 * hB:256 + 32 * hB + 32],
             w_qk_red[:, 32 * hA:32 * hA + 32] * sr,
             w_qk_red[:, 32 * hB:32 * hB + 32] * sr], 1)
        wv = np.concatenate([w_qkv_full[:, 1024 + 64 * hA:1024 + 64 * hA + 64],
                             w_qkv_full[:, 1024 + 64 * hB:1024 + 64 * hB + 64],
                             w_v_red[:, 64 * hA:64 * hA + 64],
                             w_v_red[:, 64 * hB:64 * hB + 64]], 1)
        wp = np.concatenate([w_proj[64 * hA:64 * hA + 64, :],
                             w_proj[64 * hB:64 * hB + 64, :],
                             w_proj[512 + 64 * hA:512 + 64 * hA + 64, :],
                             w_proj[512 + 64 * hB:512 + 64 * hB + 64, :]], 0)
        wqkv = np.concatenate([wq, wk, wqkr, wv], 1)
        in_maps.append({
            "xT": np.ascontiguousarray(x[b].T).astype(np.float16),
            "wqkv": np.ascontiguousarray(wqkv).astype(np.float16),
            "wproj": np.ascontiguousarray(wp).astype(np.float16),
        })
    return in_maps


def kernel(x, w_qkv_full, w_qk_red, w_v_red, w_proj):
    nc = _get_nc()
    in_maps = make_in_maps(x, w_qkv_full, w_qk_red, w_v_red, w_proj)
    r = bass_utils.run_bass_kernel_spmd(nc, in_maps,
                                        core_ids=list(range(N_CORES)),
                                        trace=False)
    outs = [r.results[c]["o"] for c in range(N_CORES)]
    y = np.zeros((B, T, C), np.float32)
    for b in range(B):
        for j in range(4):
            y[b] += np.asarray(outs[4 * b + j], np.float32)
    return y
